# revision 3
# baseline (speedup 1.0000x reference)
"""KeyFormer attention kernel for 8 TRN2 NeuronCores (Bass/Tile).

Math (per reference):
    qkv = X @ [W_q | W_k | W_v]; q,k,v split per head h (D=128)
    cache_K/V rows P:P+M replaced by k/v          (P=3584, M=512, L=4096)
    scores  = q @ K^T                             [H, M, L]
    weights = softmax(scores)  (no max-sub)
    perturb_out = softmax((scores + noise)/1.5)
    output  = (weights @ V) as [M, H*D]
Returns (output, perturb_out).

Sharding: tensor-parallel over heads — core c owns heads 4c..4c+3.
Each core computes its W_q/W_k/W_v column block (QKV projection),
both softmax paths, and its slice of the output. Host does pure
slicing/transpose/concat plus the final (tiny) output normalization.

On-device layout choices:
  - qT/kT are produced as [d, m] tiles directly (projection emits the
    transposed orientation), v is produced as [m, d] (natural V rows).
  - scores are computed twice, in [m, l] layout (noise path: matches
    noise/perturb DRAM layout, free-dim softmax via ACT accum_out) and
    in [l, m] layout (weights path: exp tiles feed the output matmul
    and a ones-vector matmul accumulates the softmax denominator).
  - all matmul operands are float32r (full-rate fp32-ish on the PE).
"""

import os
import sys

for _p in ("/opt/trn_rl_repo", "/root/.axon_site/_ro/trn_rl_repo"):
    if os.path.isdir(_p) and _p not in sys.path:
        sys.path.append(_p)

import numpy as np

M, N, D = 512, 4096, 128
H, L, P_CONST = 32, 4096, 3584
N_CORES = 8
HPC = H // N_CORES          # 4 heads per core
NCH = N // 128              # 32 contraction chunks
MT = M // 128               # 4 m-tiles
LC = L // 512               # 8 l-chunks (512 wide)
LT = L // 128               # 32 l-tiles (128 wide)
LB = P_CONST // 128         # 28 body l-tiles loaded from DRAM cache

_built = None


def _build():
    """Build + compile the SPMD Bass program once."""
    import concourse.bass as bass
    import concourse.tile as tile
    from concourse import bacc, mybir
    from contextlib import ExitStack

    F32 = mybir.dt.float32
    F32R = mybir.dt.float32r
    PSUM = bass.MemorySpace.PSUM
    EXP = mybir.ActivationFunctionType.Exp

    nc = bacc.Bacc("TRN2", target_bir_lowering=False, debug=False,
                   num_devices=N_CORES)

    xt_d = nc.dram_tensor("xt", [NCH, 128, M], F32R, kind="ExternalInput").ap()
    wq_d = nc.dram_tensor("wq", [NCH, 128, 512], F32R, kind="ExternalInput").ap()
    wk_d = nc.dram_tensor("wk", [NCH, 128, 512], F32R, kind="ExternalInput").ap()
    wv_d = nc.dram_tensor("wv", [NCH, 128, 512], F32R, kind="ExternalInput").ap()
    kt_d = nc.dram_tensor("kt", [HPC, 128, P_CONST], F32R, kind="ExternalInput").ap()
    vb_d = nc.dram_tensor("vb", [HPC, LB, 128, 128], F32R, kind="ExternalInput").ap()
    nz_d = nc.dram_tensor("nz", [HPC, M, L], F32, kind="ExternalInput").ap()
    pout_d = nc.dram_tensor("pout", [HPC, M, L], F32, kind="ExternalOutput").ap()
    outn_d = nc.dram_tensor("outn", [HPC, 128, M], F32, kind="ExternalOutput").ap()
    den_d = nc.dram_tensor("den", [HPC, 1, M], F32, kind="ExternalOutput").ap()

    with tile.TileContext(nc) as tc, ExitStack() as ctx:
        persist = ctx.enter_context(tc.tile_pool(name="persist", bufs=1))
        qt_sb = persist.tile([128, HPC, M], F32R)       # qT per head [d, m]
        kt_sb = persist.tile([128, HPC, M], F32R)       # kT per head [d, m]
        v_sb = persist.tile([128, MT, 512], F32R)       # v rows [m, 4 heads * d]
        ones_f = persist.tile([128, 1], F32)
        nc.gpsimd.memset(ones_f[:], 1.0)
        ones = persist.tile([128, 1], F32R)
        nc.vector.tensor_copy(ones[:], ones_f[:])

        # ---- Phase 1: fused QKV projection --------------------------------
        with tc.tile_pool(name="xtp", bufs=1) as xtp, \
             tc.tile_pool(name="wp", bufs=3) as wp, \
             tc.tile_pool(name="qkvps", bufs=4, space=PSUM) as qkvps:
            xt = xtp.tile([128, NCH, M], F32R)
            nc.sync.dma_start(xt[:], xt_d.rearrange("c p m -> p c m"))
            for wd, kind in ((wq_d, "q"), (wk_d, "k"), (wv_d, "v")):
                accs = [qkvps.tile([128, 512], F32, tag="acc", name=f"acc{kind}{j}")
                        for j in range(4)]
                for half in range(2):
                    wt = wp.tile([128, NCH // 2, 512], F32R, tag="w")
                    nc.sync.dma_start(
                        wt[:], wd[half * 16:(half + 1) * 16].rearrange("c p j -> p c j"))
                    for ci in range(NCH // 2):
                        ch = half * 16 + ci
                        for jt in range(4):
                            if kind == "v":
                                # v in natural [m, j] orientation
                                nc.tensor.matmul(
                                    accs[jt][:],
                                    xt[:, ch, jt * 128:(jt + 1) * 128],
                                    wt[:, ci, :],
                                    start=(ch == 0), stop=(ch == NCH - 1))
                            else:
                                # q/k transposed: [j, m]
                                nc.tensor.matmul(
                                    accs[jt][:],
                                    wt[:, ci, jt * 128:(jt + 1) * 128],
                                    xt[:, ch, :],
                                    start=(ch == 0), stop=(ch == NCH - 1))
                dest = {"q": qt_sb, "k": kt_sb, "v": v_sb}[kind]
                for jt in range(4):
                    nc.vector.tensor_copy(dest[:, jt, :], accs[jt][:])

        # ---- Phase 2: attention, head by head -----------------------------
        with tc.tile_pool(name="ktp", bufs=2) as ktp, \
             tc.tile_pool(name="vp", bufs=2) as vp, \
             tc.tile_pool(name="nzp", bufs=2) as nzp, \
             tc.tile_pool(name="awp", bufs=2) as awp, \
             tc.tile_pool(name="xptp", bufs=3) as xptp, \
             tc.tile_pool(name="smp", bufs=4) as smp, \
             tc.tile_pool(name="aps", bufs=2, space=PSUM) as aps, \
             tc.tile_pool(name="bps", bufs=3, space=PSUM) as bps, \
             tc.tile_pool(name="ops", bufs=1, space=PSUM) as ops_, \
             tc.tile_pool(name="dps", bufs=1, space=PSUM) as dps:

            kt_tiles, v_tiles = {}, {}

            def load_head(h):
                ktt = ktp.tile([128, L], F32R, tag="kt")
                nc.sync.dma_start(ktt[:, 0:P_CONST], kt_d[h])
                nc.vector.tensor_copy(ktt[:, P_CONST:L], kt_sb[:, h, :])
                vt = vp.tile([128, LT, 128], F32R, tag="v")
                nc.sync.dma_start(vt[:, 0:LB, :], vb_d[h].rearrange("c p d -> p c d"))
                for mt in range(MT):
                    nc.vector.tensor_copy(vt[:, LB + mt, :],
                                          v_sb[:, mt, h * 128:(h + 1) * 128])
                kt_tiles[h] = ktt
                v_tiles[h] = vt

            load_head(0)
            for h in range(HPC):
                if h + 1 < HPC:
                    load_head(h + 1)
                ktt = kt_tiles.pop(h)
                vt = v_tiles.pop(h)

                # noise prefetch for the A-side
                nz_tiles = []
                for mt in range(MT):
                    nzt = nzp.tile([128, L], F32, tag="nz")
                    nc.sync.dma_start(nzt[:], nz_d[h, mt * 128:(mt + 1) * 128, :])
                    nz_tiles.append(nzt)

                # B-side: scores^T -> exp -> output matmul + denominator
                outacc = ops_.tile([128, M], F32, tag="out")
                den = dps.tile([1, M], F32, tag="den")
                for lt in range(LT):
                    sc = bps.tile([128, M], F32, tag="sct")
                    nc.tensor.matmul(sc[:], ktt[:, lt * 128:(lt + 1) * 128],
                                     qt_sb[:, h, :], start=True, stop=True)
                    xpt = xptp.tile([128, M], F32R, tag="xpt")
                    nc.scalar.activation(xpt[:], sc[:], EXP)
                    nc.tensor.matmul(outacc[:], vt[:, lt, :], xpt[:],
                                     start=(lt == 0), stop=(lt == LT - 1))
                    nc.tensor.matmul(den[:], ones[:], xpt[:],
                                     start=(lt == 0), stop=(lt == LT - 1))
                osb = smp.tile([128, M], F32, tag="osb")
                nc.scalar.copy(osb[:], outacc[:])
                nc.sync.dma_start(outn_d[h], osb[:])
                dsb = smp.tile([1, M], F32, tag="dsb")
                nc.scalar.copy(dsb[:], den[:])
                nc.sync.dma_start(den_d[h], dsb[:])

                # A-side: scores -> +noise -> exp (fused rowsum) -> normalize
                for mt in range(MT):
                    nzt = nz_tiles[mt]
                    aw = awp.tile([128, L], F32, tag="aw")
                    for lc in range(LC):
                        sa = aps.tile([128, 512], F32, tag="sa")
                        nc.tensor.matmul(sa[:],
                                         qt_sb[:, h, mt * 128:(mt + 1) * 128],
                                         ktt[:, lc * 512:(lc + 1) * 512],
                                         start=True, stop=True)
                        nc.vector.tensor_add(aw[:, lc * 512:(lc + 1) * 512],
                                             sa[:], nzt[:, lc * 512:(lc + 1) * 512])
                    dn = smp.tile([128, 1], F32, tag="dn")
                    nc.scalar.activation(aw[:], aw[:], EXP, scale=1.0 / 1.5,
                                         accum_out=dn[:])
                    rc = smp.tile([128, 1], F32, tag="rc")
                    nc.vector.reciprocal(rc[:], dn[:])
                    nc.vector.tensor_scalar_mul(aw[:], aw[:], rc[:])
                    nc.sync.dma_start(pout_d[h, mt * 128:(mt + 1) * 128, :], aw[:])

    nc.compile()
    return nc


def _get_built():
    global _built
    if _built is None:
        _built = _build()
    return _built


def kernel(X, W_q, W_k, W_v, noise, cache_K, cache_V, P):
    from concourse.bass_utils import run_bass_kernel_spmd

    P = int(P)
    assert P == P_CONST, f"kernel compiled for P={P_CONST}, got {P}"
    nc = _get_built()

    X = np.asarray(X, np.float32)
    W_q = np.asarray(W_q, np.float32)
    W_k = np.asarray(W_k, np.float32)
    W_v = np.asarray(W_v, np.float32)
    noise = np.asarray(noise, np.float32)
    cache_K = np.asarray(cache_K, np.float32)
    cache_V = np.asarray(cache_V, np.float32)

    XT = np.ascontiguousarray(X.T).reshape(NCH, 128, M)
    in_maps = []
    for c in range(N_CORES):
        hs = slice(c * HPC, (c + 1) * HPC)
        cols = slice(c * 512, (c + 1) * 512)
        in_maps.append({
            "xt": XT,
            "wq": np.ascontiguousarray(W_q[:, cols]).reshape(NCH, 128, 512),
            "wk": np.ascontiguousarray(W_k[:, cols]).reshape(NCH, 128, 512),
            "wv": np.ascontiguousarray(W_v[:, cols]).reshape(NCH, 128, 512),
            "kt": np.ascontiguousarray(cache_K[hs, :P, :].transpose(0, 2, 1)),
            "vb": np.ascontiguousarray(cache_V[hs, :P, :]).reshape(HPC, LB, 128, 128),
            "nz": np.ascontiguousarray(noise[hs]),
        })

    res = run_bass_kernel_spmd(nc, in_maps, list(range(N_CORES)))
    kernel.last_results = res

    out = np.empty((M, N), np.float32)
    perturb = np.empty((H, M, L), np.float32)
    for c in range(N_CORES):
        r = res.results[c]
        perturb[c * HPC:(c + 1) * HPC] = r["pout"]
        outn = r["outn"]                  # [HPC, 128, M] unnormalized out^T
        den = r["den"][:, 0, :]           # [HPC, M]
        for i in range(HPC):
            g = c * HPC + i
            out[:, g * 128:(g + 1) * 128] = (outn[i] / den[i][None, :]).T
    return out, perturb


# revision 7
# speedup vs baseline: 2.4366x; 2.4366x over previous
"""KeyFormer attention kernel for 8 TRN2 NeuronCores (Bass/Tile).

Math (per reference):
    qkv = X @ [W_q | W_k | W_v]; q,k,v split per head h (D=128)
    cache_K/V rows P:P+M replaced by k/v          (P=3584, M=512, L=4096)
    scores  = q @ K^T                             [H, M, L]
    weights = softmax(scores)  (no max-sub)
    perturb_out = softmax((scores + noise)/1.5)
    output  = (weights @ V) as [M, H*D]
Returns (output, perturb_out).

Sharding: tensor-parallel over heads — core c owns heads 4c..4c+3.
Each core computes its W_q/W_k/W_v column block (QKV projection),
both softmax paths, and its slice of the output. Host does pure
slicing/transpose/concat plus the final (tiny) output normalization.

Layouts: qT/kT produced as [d, m] tiles directly, v as [m, d] (natural
V rows). Scores computed twice: [m, l] (noise path; free-dim softmax
via ACT accum_out) and [l, m] (weights path; exp tiles feed the output
matmul and a ones-matmul accumulates the denominator). All matmul
operands are float32r (full-rate fp32-ish on the PE).

Scheduling (v3):
  - noise + KT pools sit below the QKV pools on the SBUF stack, so the
    big noise stream and next-head KT loads never wait on the QKV pool
    release barrier; V/aw/exp pools reuse the released X^T/W space
    (V isn't needed until one exp-chain into each head).
  - W streams in 1MB eighth-tiles interleaved with X^T chunk loads.
  - input DMAs ride the sync HWDGE FIFO; output DMAs ride the idle
    GPSIMD SWDGE queue so compute-gated outputs can't head-of-line
    block the input stream.
  - per head, B-side (weights path) l-tile octets interleave with
    A-side (noise path) m-tiles so exp work and writeback flow steadily.
  - V-tail copies are issued after the Wv projection writes v_sb
    (program order = data order in Tile).
"""

import os
import sys

for _p in ("/opt/trn_rl_repo", "/root/.axon_site/_ro/trn_rl_repo"):
    if os.path.isdir(_p) and _p not in sys.path:
        sys.path.append(_p)

import numpy as np

M, N, D = 512, 4096, 128
H, L, P_CONST = 32, 4096, 3584
N_CORES = 8
HPC = H // N_CORES          # 4 heads per core
NCH = N // 128              # 32 contraction chunks
MT = M // 128               # 4 m-tiles
LC = L // 512               # 8 l-chunks (512 wide)
LT = L // 128               # 32 l-tiles (128 wide)
LB = P_CONST // 128         # 28 body l-tiles loaded from DRAM cache
NW = 8                      # W streamed in eighths of 4 chunks
WCH = NCH // NW

_built = None


def _build():
    """Build + compile the SPMD Bass program once."""
    import concourse.bass as bass
    import concourse.tile as tile
    from concourse import bacc, mybir
    from contextlib import ExitStack

    F32 = mybir.dt.float32
    F32R = mybir.dt.float32r
    PSUM = bass.MemorySpace.PSUM
    EXP = mybir.ActivationFunctionType.Exp

    nc = bacc.Bacc("TRN2", target_bir_lowering=False, debug=False,
                   num_devices=N_CORES)

    xt_d = nc.dram_tensor("xt", [NCH, 128, M], F32R, kind="ExternalInput").ap()
    wq_d = nc.dram_tensor("wq", [NCH, 128, 512], F32R, kind="ExternalInput").ap()
    wk_d = nc.dram_tensor("wk", [NCH, 128, 512], F32R, kind="ExternalInput").ap()
    wv_d = nc.dram_tensor("wv", [NCH, 128, 512], F32R, kind="ExternalInput").ap()
    kt_d = nc.dram_tensor("kt", [HPC, 128, P_CONST], F32R, kind="ExternalInput").ap()
    vb_d = nc.dram_tensor("vb", [HPC, LB, 128, 128], F32R, kind="ExternalInput").ap()
    nz_d = nc.dram_tensor("nz", [HPC, M, L], F32, kind="ExternalInput").ap()
    pout_d = nc.dram_tensor("pout", [HPC, M, L], F32, kind="ExternalOutput").ap()
    outn_d = nc.dram_tensor("outn", [HPC, 128, M], F32, kind="ExternalOutput").ap()
    den_d = nc.dram_tensor("den", [HPC, 1, M], F32, kind="ExternalOutput").ap()

    with tile.TileContext(nc) as tc, ExitStack() as ctx:
        persist = ctx.enter_context(tc.tile_pool(name="persist", bufs=1))
        qt_sb = persist.tile([128, HPC, M], F32R)       # qT per head [d, m]
        kt_sb = persist.tile([128, HPC, M], F32R)       # kT per head [d, m]
        v_sb = persist.tile([128, MT, 512], F32R)       # v rows [m, 4 heads * d]
        ones_f = persist.tile([128, 1], F32)
        nc.gpsimd.memset(ones_f[:], 1.0)
        ones = persist.tile([128, 1], F32R)
        nc.vector.tensor_copy(ones[:], ones_f[:])

        # Early pools: below the QKV stack so they never wait on its release.
        nzp = ctx.enter_context(tc.tile_pool(name="nzp", bufs=2))
        ktp = ctx.enter_context(tc.tile_pool(name="ktp", bufs=2))
        bps = ctx.enter_context(tc.tile_pool(name="bps", bufs=2, space=PSUM))
        ops_ = ctx.enter_context(tc.tile_pool(name="ops", bufs=1, space=PSUM))
        dps = ctx.enter_context(tc.tile_pool(name="dps", bufs=1, space=PSUM))

        kt_tiles = {}

        def load_kt(h):
            """KT body DMA + kT tail copy (kt_sb must already be written)."""
            ktt = ktp.tile([128, L], F32R, tag="kt", name=f"ktt{h}")
            nc.sync.dma_start(ktt[:, 0:P_CONST], kt_d[h])
            nc.vector.tensor_copy(ktt[:, P_CONST:L], kt_sb[:, h, :])
            kt_tiles[h] = ktt

        # ---- Phase 1: fused QKV projection --------------------------------
        with tc.tile_pool(name="xtp", bufs=1) as xtp, \
             tc.tile_pool(name="wp", bufs=3) as wp, \
             tc.tile_pool(name="qkvps", bufs=4, space=PSUM) as qkvps:
            xt = xtp.tile([128, NCH, M], F32R)
            for wi, (wd, kind) in enumerate(((wq_d, "q"), (wk_d, "k"), (wv_d, "v"))):
                accs = [qkvps.tile([128, 512], F32, tag="acc", name=f"acc{kind}{j}")
                        for j in range(4)]
                for q in range(NW):
                    if wi == 0 and q % 2 == 0:
                        g = q // 2
                        nc.sync.dma_start(
                            xt[:, g * 8:(g + 1) * 8, :],
                            xt_d[g * 8:(g + 1) * 8].rearrange("c p m -> p c m"))
                    wt = wp.tile([128, WCH, 512], F32R, tag="w",
                                 name=f"w{kind}{q}")
                    nc.sync.dma_start(
                        wt[:], wd[q * WCH:(q + 1) * WCH].rearrange("c p j -> p c j"))
                    for ci in range(WCH):
                        ch = q * WCH + ci
                        for jt in range(4):
                            if kind == "v":
                                # v in natural [m, j] orientation
                                nc.tensor.matmul(
                                    accs[jt][:],
                                    xt[:, ch, jt * 128:(jt + 1) * 128],
                                    wt[:, ci, :],
                                    start=(ch == 0), stop=(ch == NCH - 1))
                            else:
                                # q/k transposed: [j, m]
                                nc.tensor.matmul(
                                    accs[jt][:],
                                    wt[:, ci, jt * 128:(jt + 1) * 128],
                                    xt[:, ch, :],
                                    start=(ch == 0), stop=(ch == NCH - 1))
                dest = {"q": qt_sb, "k": kt_sb, "v": v_sb}[kind]
                for jt in range(4):
                    nc.vector.tensor_copy(dest[:, jt, :], accs[jt][:])
                if kind == "k":
                    load_kt(0)          # overlaps the Wv phase
            load_kt(1)

        # ---- Phase 2: attention, head by head -----------------------------
        with tc.tile_pool(name="vp", bufs=2) as vp, \
             tc.tile_pool(name="awp", bufs=2) as awp, \
             tc.tile_pool(name="xptp", bufs=3) as xptp, \
             tc.tile_pool(name="smp", bufs=4) as smp, \
             tc.tile_pool(name="aps", bufs=2, space=PSUM) as aps:

            v_tiles = {}

            def load_vt(h):
                vt = vp.tile([128, LT, 128], F32R, tag="v", name=f"vt{h}")
                nc.sync.dma_start(vt[:, 0:LB, :],
                                  vb_d[h].rearrange("c p d -> p c d"))
                for mt in range(MT):
                    nc.vector.tensor_copy(vt[:, LB + mt, :],
                                          v_sb[:, mt, h * 128:(h + 1) * 128])
                v_tiles[h] = vt

            load_vt(0)
            for h in range(HPC):
                if h + 2 < HPC:
                    load_kt(h + 2)
                if h + 1 < HPC:
                    load_vt(h + 1)
                ktt = kt_tiles.pop(h)
                vt = v_tiles.pop(h)

                # noise stream for the A-side (sync FIFO, ahead of need)
                nz_tiles = []
                for mt in range(MT):
                    nzt = nzp.tile([128, L], F32, tag="nz", name=f"nz{h}_{mt}")
                    nc.sync.dma_start(nzt[:], nz_d[h, mt * 128:(mt + 1) * 128, :])
                    nz_tiles.append(nzt)

                outacc = ops_.tile([128, M], F32, tag="out", name=f"outacc{h}")
                den = dps.tile([1, M], F32, tag="den", name=f"den{h}")

                for mt in range(MT):
                    # B-side octet: scores^T -> exp -> out matmul + denominator
                    for lt in range(mt * 8, mt * 8 + 8):
                        sc = bps.tile([128, M], F32, tag="sct", name=f"sc{h}_{lt}")
                        nc.tensor.matmul(sc[:], ktt[:, lt * 128:(lt + 1) * 128],
                                         qt_sb[:, h, :], start=True, stop=True)
                        xpt = xptp.tile([128, M], F32R, tag="xpt",
                                        name=f"xpt{h}_{lt}")
                        nc.scalar.activation(xpt[:], sc[:], EXP)
                        nc.tensor.matmul(outacc[:], vt[:, lt, :], xpt[:],
                                         start=(lt == 0), stop=(lt == LT - 1))
                        nc.tensor.matmul(den[:], ones[:], xpt[:],
                                         start=(lt == 0), stop=(lt == LT - 1))
                    # A-side m-tile: scores -> +noise -> exp (rowsum) -> norm
                    nzt = nz_tiles[mt]
                    aw = awp.tile([128, L], F32, tag="aw", name=f"aw{h}_{mt}")
                    for lc in range(LC):
                        sa = aps.tile([128, 512], F32, tag="sa",
                                      name=f"sa{h}_{mt}_{lc}")
                        nc.tensor.matmul(sa[:],
                                         qt_sb[:, h, mt * 128:(mt + 1) * 128],
                                         ktt[:, lc * 512:(lc + 1) * 512],
                                         start=True, stop=True)
                        nc.vector.tensor_add(aw[:, lc * 512:(lc + 1) * 512],
                                             sa[:], nzt[:, lc * 512:(lc + 1) * 512])
                    dn = smp.tile([128, 1], F32, tag="dn", name=f"dn{h}_{mt}")
                    nc.scalar.activation(aw[:], aw[:], EXP, scale=1.0 / 1.5,
                                         accum_out=dn[:])
                    rc = smp.tile([128, 1], F32, tag="rc", name=f"rc{h}_{mt}")
                    nc.vector.reciprocal(rc[:], dn[:])
                    nc.vector.tensor_scalar_mul(aw[:], aw[:], rc[:])
                    nc.gpsimd.dma_start(pout_d[h, mt * 128:(mt + 1) * 128, :],
                                        aw[:])

                osb = smp.tile([128, M], F32, tag="osb", bufs=2, name=f"osb{h}")
                nc.scalar.copy(osb[:], outacc[:])
                nc.gpsimd.dma_start(outn_d[h], osb[:])
                dsb = smp.tile([1, M], F32, tag="dsb", bufs=2, name=f"dsb{h}")
                nc.scalar.copy(dsb[:], den[:])
                nc.gpsimd.dma_start(den_d[h], dsb[:])

    nc.compile()
    return nc


def _get_built():
    global _built
    if _built is None:
        _built = _build()
    return _built


def kernel(X, W_q, W_k, W_v, noise, cache_K, cache_V, P):
    from concourse.bass_utils import run_bass_kernel_spmd

    P = int(P)
    assert P == P_CONST, f"kernel compiled for P={P_CONST}, got {P}"
    nc = _get_built()

    X = np.asarray(X, np.float32)
    W_q = np.asarray(W_q, np.float32)
    W_k = np.asarray(W_k, np.float32)
    W_v = np.asarray(W_v, np.float32)
    noise = np.asarray(noise, np.float32)
    cache_K = np.asarray(cache_K, np.float32)
    cache_V = np.asarray(cache_V, np.float32)

    XT = np.ascontiguousarray(X.T).reshape(NCH, 128, M)
    in_maps = []
    for c in range(N_CORES):
        hs = slice(c * HPC, (c + 1) * HPC)
        cols = slice(c * 512, (c + 1) * 512)
        in_maps.append({
            "xt": XT,
            "wq": np.ascontiguousarray(W_q[:, cols]).reshape(NCH, 128, 512),
            "wk": np.ascontiguousarray(W_k[:, cols]).reshape(NCH, 128, 512),
            "wv": np.ascontiguousarray(W_v[:, cols]).reshape(NCH, 128, 512),
            "kt": np.ascontiguousarray(cache_K[hs, :P, :].transpose(0, 2, 1)),
            "vb": np.ascontiguousarray(cache_V[hs, :P, :]).reshape(HPC, LB, 128, 128),
            "nz": np.ascontiguousarray(noise[hs]),
        })

    res = run_bass_kernel_spmd(nc, in_maps, list(range(N_CORES)))
    kernel.last_results = res

    out = np.empty((M, N), np.float32)
    perturb = np.empty((H, M, L), np.float32)
    for c in range(N_CORES):
        r = res.results[c]
        perturb[c * HPC:(c + 1) * HPC] = r["pout"]
        outn = r["outn"]                  # [HPC, 128, M] unnormalized out^T
        den = r["den"][:, 0, :]           # [HPC, M]
        for i in range(HPC):
            g = c * HPC + i
            out[:, g * 128:(g + 1) * 128] = (outn[i] / den[i][None, :]).T
    return out, perturb
